# revision 22
# baseline (speedup 1.0000x reference)
"""BitSPPF kernel for Trainium2 (8 NeuronCores, data-parallel over batch).

Pipeline per core (4 images):
  cv1 (1x1 ternary conv; k-subtiles 0-5 in fp8 DoubleRow, 6-7 in bf16 for
  precision) -> BN+SiLU (ACT) -> 3x chained 5x5 maxpool (separable max
  trees on DVE, bf16, two channel-tiles per instruction) -> centered fp8
  re-encode (ACT) -> cv2 in fp8 DoubleRow -> BN+SiLU (fused over both
  psum banks) -> DRAM.

Schedule: per-image "windows" paced by the DVE pool chain. Window b runs
pools(b)+encodes(b) on DVE/ACT while the PE executes cv2(b-1) compact
(all 16 half-units; its v8 landed at the end of window b-1) plus
cv1(b+1) pulled one image ahead. cv1(0) runs pre-loop in two mt-pair
passes so pair-0 pools start after ~half of cv1(0). The final image's
cv2 pre-streams y3-independent k-pairs four psum tiles deep (borrowing
cv1's idle ps1 banks) while the last pool chain drains.

cv2's fp8 precision is recovered by per-channel mean centering with the
exact bias correction W@c folded into cv2's BN bias (host-side).
"""

import os
import sys

for _p in ("/opt/trn_rl_repo",):
    if _p not in sys.path and os.path.isdir(_p):
        sys.path.insert(0, _p)

import numpy as np
import ml_dtypes

import concourse.bass as bass
import concourse.tile as tile
from concourse import bacc, mybir

BF16 = mybir.dt.bfloat16
F32 = mybir.dt.float32
FP8 = mybir.dt.float8e4
NPBF16 = ml_dtypes.bfloat16
NPFP8 = ml_dtypes.float8_e4m3

# Problem shapes (hardcoded per spec)
B, C1, H, W = 32, 1024, 40, 40
HID, C2 = 512, 1024
S = H * W  # 1600
N_CORES = 8
BL = B // N_CORES  # images per core

NEG = -3.0e38  # effectively -inf for maxpool padding, finite in bf16

EPS = 1e-8
BN_EPS = 1e-5

DR = mybir.MatmulPerfMode.DoubleRow

# cv1 precision split: first NFP8 input channels via fp8 DoubleRow,
# remainder in bf16. 768 keeps the end-to-end max error ~0.016 (<2e-2).
NFP8 = 768
KT8 = NFP8 // 128          # 6 fp8 k-subtiles
KP8 = KT8 // 2             # 3 DoubleRow pairs
KT16 = (C1 - NFP8) // 128  # 2 bf16 k-tiles

MT1 = HID // 128      # 4 m-tiles (= pool channel tiles)
KT2 = 4 * HID // 128  # 16 k-subtiles for cv2
KP2 = KT2 // 2        # 8 fp8 DoubleRow pairs
MT2 = C2 // 128       # 8 m-tiles for cv2
NQ = 4                # spatial quarters (10 rows of 40)
QW = S // NQ          # 400


def _pools_chain(nc, P, HX, M2, Pout, padded_out, r0=0, r1=40):
    """5x5 stride-1 pad-2 maxpool over a channel-tile PAIR for output rows
    [r0, r1): P -> Pout.

    P: [128, 2, 40, 44] bf16, data in cols 2..41, cols {0,1,42,43} = NEG.
    HX: [128, 2, 44, 40] scratch; rows {0,1,42,43} pre-set to NEG.
    M2: [128, 2, 43, 44] scratch.
    Pout: [128, 2, 40, 44] (padded_out) or [128, 2, 40, 40].
    """
    ri0, ri1 = max(0, r0 - 2), min(40, r1 + 2)
    nc.vector.tensor_max(M2[:, :, ri0:ri1, 0:42], P[:, :, ri0:ri1, 0:42],
                         P[:, :, ri0:ri1, 1:43])
    nc.vector.tensor_max(HX[:, :, ri0 + 2:ri1 + 2, :], M2[:, :, ri0:ri1, 0:40],
                         M2[:, :, ri0:ri1, 2:42])
    nc.vector.tensor_max(HX[:, :, ri0 + 2:ri1 + 2, :], HX[:, :, ri0 + 2:ri1 + 2, :],
                         P[:, :, ri0:ri1, 4:44])
    nc.vector.tensor_max(M2[:, :, r0:r1 + 2, 0:40], HX[:, :, r0:r1 + 2, :],
                         HX[:, :, r0 + 1:r1 + 3, :])
    if padded_out:
        ov = Pout[:, :, r0:r1, 2:42]
    else:
        ov = Pout[:, :, r0:r1, :]
    nc.vector.tensor_max(ov, M2[:, :, r0:r1, 0:40], M2[:, :, r0 + 2:r1 + 2, 0:40])
    nc.vector.tensor_max(ov, ov, HX[:, :, r0 + 4:r1 + 4, :])


def _build_nc(bl=BL):
    nc = bacc.Bacc(trn_type="TRN2", debug=False)

    xq8_d = nc.dram_tensor("xq8", [bl, NQ, 128, KT8, QW], FP8,
                           kind="ExternalInput")
    xq16_d = nc.dram_tensor("xq16", [bl, NQ, 128, KT16, QW], BF16,
                            kind="ExternalInput")
    w1t8_d = nc.dram_tensor("w1t8", [128, KT8, HID], FP8, kind="ExternalInput")
    w1t16_d = nc.dram_tensor("w1t16", [128, KT16, HID], BF16,
                             kind="ExternalInput")
    w2t_d = nc.dram_tensor("w2t", [128, KT2, C2], FP8, kind="ExternalInput")
    sc1_d = nc.dram_tensor("sc1", [HID], F32, kind="ExternalInput")
    bi1_d = nc.dram_tensor("bi1", [HID], F32, kind="ExternalInput")
    sc2_d = nc.dram_tensor("sc2", [C2], F32, kind="ExternalInput")
    bi2_d = nc.dram_tensor("bi2", [C2], F32, kind="ExternalInput")
    cng_d = nc.dram_tensor("cng", [16 * 128], F32, kind="ExternalInput")
    out_d = nc.dram_tensor("out", [bl, C2, S], F32, kind="ExternalOutput")

    xv8 = xq8_d.ap()
    xv16 = xq16_d.ap()
    ov = out_d.ap().rearrange("b (mt p) s -> b p mt s", p=128)

    # CoreSim doesn't implement Silu; allow substituting Sigmoid for
    # wiring-validation sim runs (numerics then differ by design).
    if os.environ.get("BITSPPF_SIM_ACT") == "sigmoid":
        silu = mybir.ActivationFunctionType.Sigmoid
    else:
        silu = mybir.ActivationFunctionType.Silu
    ident = mybir.ActivationFunctionType.Identity

    with tile.TileContext(nc) as tc:
        with (
            tc.tile_pool(name="const", bufs=1) as const,
            tc.tile_pool(name="xin8", bufs=4) as xin8,
            tc.tile_pool(name="xin16", bufs=4) as xin16,
            tc.tile_pool(name="pbuf0", bufs=4) as pbuf0,
            tc.tile_pool(name="plad", bufs=6) as plad,
            tc.tile_pool(name="v8p", bufs=3) as v8p,
            tc.tile_pool(name="work", bufs=1) as work,
            tc.tile_pool(name="osb", bufs=3) as osb,
            tc.tile_pool(name="ps1", bufs=2, space="PSUM") as ps1p,
            tc.tile_pool(name="ps2", bufs=3, space="PSUM") as ps2p,
        ):
            # Pre-warm the ACT engine's Silu spline tables (~2.7us load)
            # during the initial DMA window instead of at the first real
            # activation. The PE HAM warm-up runs on a memset tile so it
            # starts immediately (no DMA dependency) and has the clock gate
            # at 8/8 when the first real matmul issues.
            warm = const.tile([128, 130], BF16)
            nc.vector.memset(warm, 0.0)
            warmf = const.tile([128, 2], F32)
            nc.vector.memset(warmf, 0.0)
            nc.scalar.activation(out=warmf, in_=warmf, func=silu)
            wps = ps1p.tile([128, 512], F32, tag="ps1")
            for _i in range(60):
                nc.tensor.matmul(
                    wps[0:4, 0:4], warm[:, 0:4], warm[:, 0:4],
                    start=True, stop=True,
                )
            for _i in range(20):
                nc.tensor.matmul(
                    wps[:, 0:32], warm[:, 0:128], warm[:, 0:32],
                    start=True, stop=True,
                )

            sc1_sb = const.tile([128, MT1], F32)
            nc.scalar.dma_start(sc1_sb, sc1_d.ap().rearrange("(t p) -> p t", p=128))
            bi1_sb = const.tile([128, MT1], F32)
            nc.scalar.dma_start(bi1_sb, bi1_d.ap().rearrange("(t p) -> p t", p=128))
            sc2_sb = const.tile([128, MT2], F32)
            nc.scalar.dma_start(sc2_sb, sc2_d.ap().rearrange("(t p) -> p t", p=128))
            bi2_sb = const.tile([128, MT2], F32)
            nc.scalar.dma_start(bi2_sb, bi2_d.ap().rearrange("(t p) -> p t", p=128))
            cng_sb = const.tile([128, 16], F32)
            nc.scalar.dma_start(cng_sb, cng_d.ap().rearrange("(t p) -> p t", p=128))

            xh8 = {}   # (b, q) -> x8 quarter tile [128, KT8, 400]
            xh16 = {}  # (b, q) -> x16 quarter tile [128, KT16, 400]

            def load_x_quarter(b, q):
                t8 = xin8.tile([128, KT8, QW], FP8, tag="x8")
                nc.sync.dma_start(t8, xv8[b][q])
                t16 = xin16.tile([128, KT16, QW], BF16, tag="x16")
                nc.sync.dma_start(t16, xv16[b][q])
                xh8[(b, q)] = t8
                xh16[(b, q)] = t16

            load_x_quarter(0, 0)
            load_x_quarter(0, 1)

            w1_sb8 = const.tile([128, KT8, HID], FP8)
            nc.gpsimd.dma_start(w1_sb8, w1t8_d.ap())
            w1_sb16 = const.tile([128, KT16, HID], BF16)
            nc.gpsimd.dma_start(w1_sb16, w1t16_d.ap())

            def load_w2():
                w2_sb = const.tile([128, KT2, C2], FP8)
                nc.sync.dma_start(w2_sb, w2t_d.ap())
                return w2_sb

            pimg = {}  # b -> [P0 pair list, V8]

            def cv1_alloc(b):
                v8h = v8p.tile([128, MT1, 40, 40], FP8, tag="V8H", bufs=3)
                pimg[b] = [[], v8h, None]
                for _pr in range(2):
                    P0 = pbuf0.tile([128, 2, 40, 44], BF16, tag="P0")
                    nc.gpsimd.memset(P0[:, :, :, 0:2], NEG)
                    nc.gpsimd.memset(P0[:, :, :, 42:44], NEG)
                    pimg[b][0].append(P0)

            def alloc_v8y(b):
                v8y = v8p.tile([128, KT2 - MT1, 40, 40], FP8,
                               tag="V8Y", bufs=2)
                pimg[b][2] = v8y

            def cv1_quarter(b, q, mts=(0, 1, 2, 3)):
                """Matmuls/activations for quarter q of image b (x quarter
                tiles must have been loaded via load_x_quarter)."""
                P0s = pimg[b][0]
                xs8 = xh8[(b, q)]
                xs16 = xh16[(b, q)]
                for mt in mts:
                    ps = ps1p.tile([128, 512], F32, tag="ps1")
                    for kp in range(KP8):
                        nc.tensor.matmul(
                            ps[:, :QW],
                            w1_sb8[:, 2 * kp:2 * kp + 2, mt * 128:(mt + 1) * 128],
                            xs8[:, 2 * kp:2 * kp + 2, :],
                            start=(kp == 0),
                            stop=False,
                            perf_mode=DR,
                        )
                    for kt in range(KT16):
                        nc.tensor.matmul(
                            ps[:, :QW],
                            w1_sb16[:, kt, mt * 128:(mt + 1) * 128],
                            xs16[:, kt, :],
                            start=False,
                            stop=(kt == KT16 - 1),
                        )
                    nc.scalar.activation(
                        out=P0s[mt // 2][:, mt % 2, q * 10:(q + 1) * 10, 2:42],
                        in_=ps[:, :QW],
                        func=silu,
                        bias=bi1_sb[:, mt:mt + 1],
                        scale=sc1_sb[:, mt:mt + 1],
                    )

            def cv1_henc_ct(b, ct):
                """fp8 re-encode of one h channel-tile into the v8 h-block."""
                P0s, v8h = pimg[b][0], pimg[b][1]
                nc.scalar.activation(
                    out=v8h[:, ct], in_=P0s[ct // 2][:, ct % 2, :, 2:42],
                    func=ident, bias=cng_sb[:, ct:ct + 1],
                )

            def cv1_henc(b):
                for ct in range(MT1):
                    cv1_henc_ct(b, ct)

            def emit_chain_level(b, pr, src, padded_out, po=None, r0=0, r1=40):
                """Maxpool level rows [r0, r1) for ct pair pr; returns the
                pair tile (pass po to continue filling an earlier slab's
                output tile)."""
                HX = work.tile([128, 2, 44, 40], BF16, tag="HX", bufs=2)
                M2 = work.tile([128, 2, 43, 44], BF16, tag="M2", bufs=2)
                if r0 == 0:
                    nc.gpsimd.memset(HX[:, :, 0:2, :], NEG)
                if r1 == 40:
                    nc.gpsimd.memset(HX[:, :, 42:44, :], NEG)
                if po is None:
                    po = plad.tile([128, 2, 40, 44], BF16, tag="PL")
                    if padded_out:
                        nc.gpsimd.memset(po[:, :, :, 0:2], NEG)
                        nc.gpsimd.memset(po[:, :, :, 42:44], NEG)
                if padded_out:
                    _pools_chain(nc, src, HX, M2, po, True, r0, r1)
                else:
                    _pools_chain(nc, src, HX, M2, po[:, :, :, 0:40], False, r0, r1)
                return po

            def emit_enc(b, pr, lvl, pl, r0=0, r1=40):
                """fp8 re-encode of pool level lvl (1..3) rows [r0, r1) for
                ct pair pr."""
                v8y = pimg[b][2]
                src = pl[:, :, :, 2:42] if lvl < 3 else pl[:, :, :, 0:40]
                for i in range(2):
                    ct = 2 * pr + i
                    j = lvl * MT1 + ct
                    nc.scalar.activation(
                        out=v8y[:, j - MT1, r0:r1, :], in_=src[:, i, r0:r1, :],
                        func=ident, bias=cng_sb[:, j:j + 1],
                    )

            def emit_cv2_half_mm(b, mt2, h, pqs, kps):
                v8h, v8y = pimg[b][1], pimg[b][2]
                for kp in kps:
                    lhs = w2_sb[:, 2 * kp:2 * kp + 2, mt2 * 128:(mt2 + 1) * 128]
                    st = kp == 0
                    sp = kp == KP2 - 1
                    if kp < 2:
                        vt, k0 = v8h, 2 * kp
                    else:
                        vt, k0 = v8y, 2 * kp - MT1
                    for j in range(2):
                        nt = 2 * h + j
                        nc.tensor.matmul(
                            pqs[j], lhs,
                            vt[:, k0:k0 + 2, nt * 10:(nt + 1) * 10, :],
                            start=st, stop=sp, perf_mode=DR,
                        )

            def cv2_unit_start(b, u):
                """Allocate psum + run kp0-5 (y3-independent) for unit u."""
                mt2, h = u // 2, u % 2
                psU = ps2p.tile([128, 2, 512], F32, tag="ps2")
                pqs = [psU[:, 0, :QW], psU[:, 1, :QW]]
                emit_cv2_half_mm(b, mt2, h, pqs, range(6))
                return psU, pqs

            def cv2_unit_finish(b, u, psU, pqs):
                """kp6-7 accumulation + fused act + one DMA for unit u."""
                mt2, h = u // 2, u % 2
                emit_cv2_half_mm(b, mt2, h, pqs, (6, 7))
                oo = osb.tile([128, 2, QW], F32, tag="o")
                nc.scalar.activation(
                    out=oo, in_=psU[:, :, :QW], func=silu,
                    bias=bi2_sb[:, mt2:mt2 + 1],
                    scale=sc2_sb[:, mt2:mt2 + 1],
                )
                nc.sync.dma_start(
                    ov[b][:, mt2, h * 2 * QW:(h + 1) * 2 * QW],
                    oo,
                )

            def emit_cv2_unit(b, u):
                psU, pqs = cv2_unit_start(b, u)
                cv2_unit_finish(b, u, psU, pqs)

            # --- image 0 pre-loop: two mt-pair passes so pair-0 pools can
            # start after roughly half of cv1(0).
            cv1_alloc(0)
            cv1_quarter(0, 0, mts=(0, 1))
            load_x_quarter(0, 2)
            cv1_quarter(0, 1, mts=(0, 1))
            load_x_quarter(0, 3)
            cv1_quarter(0, 2, mts=(0, 1))
            cv1_quarter(0, 3, mts=(0, 1))
            for q in range(NQ):
                cv1_quarter(0, q, mts=(2, 3))
            cv1_henc(0)
            w2_sb = load_w2()

            # --- steady windows: window b = pools(b)+enc(b) | cv2(b-1) | cv1(b+1)
            # Units 0-2 of each window's cv2 are pre-started (kp0-5, which
            # need only h/y1/y2) at the END of the previous window, filling
            # the PE while that window's ACT queue drains; their kp6/7+act
            # run at window start once the y3 encodes have landed. h-encodes
            # are spread through the window to avoid a pinned ACT bunch at
            # the boundary.
            held = []
            for b in range(bl - 1):
                c = b - 1
                n = b + 1

                def U(*us, _c=c):
                    if _c >= 0:
                        for u in us:
                            emit_cv2_unit(_c, u)

                P0s = pimg[b][0]
                alloc_v8y(b)
                pA1 = emit_chain_level(b, 0, P0s[0], True)
                for u, psU, pqs in held:
                    cv2_unit_finish(c, u, psU, pqs)
                cv1_alloc(n)
                load_x_quarter(n, 0)
                load_x_quarter(n, 1)
                cv1_quarter(n, 0)
                if b >= 1:
                    cv1_henc_ct(b, 0)
                emit_enc(b, 0, 1, pA1)
                U(3, 4)
                pB1 = emit_chain_level(b, 1, P0s[1], True)
                cv1_quarter(n, 1)
                if b >= 1:
                    cv1_henc_ct(b, 1)
                emit_enc(b, 1, 1, pB1)
                U(5, 6)
                pA2 = emit_chain_level(b, 0, pA1, True)
                load_x_quarter(n, 2)
                load_x_quarter(n, 3)
                cv1_quarter(n, 2)
                if b >= 1:
                    cv1_henc_ct(b, 2)
                emit_enc(b, 0, 2, pA2)
                U(7, 8)
                pB2 = emit_chain_level(b, 1, pB1, True)
                cv1_quarter(n, 3)
                if b >= 1:
                    cv1_henc_ct(b, 3)
                emit_enc(b, 1, 2, pB2)
                U(9, 10)
                pA3 = emit_chain_level(b, 0, pA2, False)
                U(11, 12)
                pB3 = emit_chain_level(b, 1, pB2, False)
                U(13, 14, 15)
                emit_enc(b, 0, 3, pA3)
                emit_enc(b, 1, 3, pB3)
                held = [(u,) + cv2_unit_start(b, u) for u in (0, 1, 2)]

            # --- final window: pools of the last image run in two spatial
            # slabs (rows 0..19 + margins, then the rest) so the last image's
            # cv2 h=0 units overlap the second slab; cv2(bl-2) interleaves as
            # usual.
            b = bl - 1
            c = b - 1
            P0s = pimg[b][0]
            alloc_v8y(b)
            # slab 0: L1 rows [0,24), L2 [0,22), L3 [0,20)
            pA1 = emit_chain_level(b, 0, P0s[0], True, r0=0, r1=24)
            for u, psU, pqs in held:
                cv2_unit_finish(c, u, psU, pqs)
            cv1_henc_ct(b, 0)
            emit_enc(b, 0, 1, pA1, 0, 20)
            pB1 = emit_chain_level(b, 1, P0s[1], True, r0=0, r1=24)
            emit_cv2_unit(c, 3)
            emit_cv2_unit(c, 4)
            cv1_henc_ct(b, 1)
            emit_enc(b, 1, 1, pB1, 0, 20)
            pA2 = emit_chain_level(b, 0, pA1, True, r0=0, r1=22)
            emit_cv2_unit(c, 5)
            emit_cv2_unit(c, 6)
            cv1_henc_ct(b, 2)
            emit_enc(b, 0, 2, pA2, 0, 20)
            pB2 = emit_chain_level(b, 1, pB1, True, r0=0, r1=22)
            emit_cv2_unit(c, 7)
            emit_cv2_unit(c, 8)
            cv1_henc_ct(b, 3)
            emit_enc(b, 1, 2, pB2, 0, 20)
            pA3 = emit_chain_level(b, 0, pA2, False, r0=0, r1=20)
            emit_cv2_unit(c, 9)
            emit_cv2_unit(c, 10)
            emit_enc(b, 0, 3, pA3, 0, 20)
            pB3 = emit_chain_level(b, 1, pB2, False, r0=0, r1=20)
            emit_cv2_unit(c, 11)
            emit_cv2_unit(c, 12)
            emit_enc(b, 1, 3, pB3, 0, 20)
            # slab 1: L1 rows [24,40), L2 [22,40), L3 [20,40)
            emit_chain_level(b, 0, P0s[0], True, po=pA1, r0=24, r1=40)
            emit_cv2_unit(c, 13)
            emit_enc(b, 0, 1, pA1, 20, 40)
            emit_chain_level(b, 1, P0s[1], True, po=pB1, r0=24, r1=40)
            emit_cv2_unit(c, 14)
            emit_cv2_unit(c, 15)
            emit_enc(b, 1, 1, pB1, 20, 40)
            emit_chain_level(b, 0, pA1, True, po=pA2, r0=22, r1=40)
            # h=0 units of the last image: all inputs (v8 rows 0..19) ready
            emit_cv2_unit(b, 0)
            emit_cv2_unit(b, 2)
            emit_enc(b, 0, 2, pA2, 20, 40)
            emit_chain_level(b, 1, pB1, True, po=pB2, r0=22, r1=40)
            emit_cv2_unit(b, 4)
            emit_cv2_unit(b, 6)
            emit_enc(b, 1, 2, pB2, 20, 40)
            emit_chain_level(b, 0, pA2, False, po=pA3, r0=20, r1=40)
            emit_cv2_unit(b, 8)
            emit_cv2_unit(b, 10)
            emit_enc(b, 0, 3, pA3, 20, 40)
            emit_chain_level(b, 1, pB2, False, po=pB3, r0=20, r1=40)
            emit_cv2_unit(b, 12)
            emit_cv2_unit(b, 14)
            emit_enc(b, 1, 3, pB3, 20, 40)
            # h=1 units after the second slab's encodes
            for u in (1, 3, 5, 7, 9, 11, 13, 15):
                emit_cv2_unit(b, u)

    nc.compile()
    return nc


_NC_CACHE = {}


def _get_nc(bl=BL):
    if bl not in _NC_CACHE:
        _NC_CACHE[bl] = _build_nc(bl)
    return _NC_CACHE[bl]


def _maxpool5_np(x):
    """x: [C, H, W] f32 -> 5x5 stride-1 pad-2 maxpool."""
    C, HH, WW = x.shape
    xp = np.full((C, HH + 4, WW + 4), -np.inf, np.float32)
    xp[:, 2:-2, 2:-2] = x
    out = np.full((C, HH, WW), -np.inf, np.float32)
    for dy in range(5):
        for dx in range(5):
            np.maximum(out, xp[:, dy:dy + HH, dx:dx + WW], out=out)
    return out


def _prep(inputs):
    """Host-side: quantize weights to ternary, fold BitNet scale + BN into
    per-channel (scale, bias), build the fp8 weights/inputs and the
    per-channel centering constants + bias correction."""
    x = np.asarray(inputs["x"], dtype=np.float32)
    w1 = np.asarray(inputs["w1"], dtype=np.float32)
    w2 = np.asarray(inputs["w2"], dtype=np.float32)
    g1 = np.asarray(inputs["g1"], dtype=np.float32)
    b1 = np.asarray(inputs["b1"], dtype=np.float32)
    m1 = np.asarray(inputs["m1"], dtype=np.float32)
    v1 = np.asarray(inputs["v1"], dtype=np.float32)
    g2 = np.asarray(inputs["g2"], dtype=np.float32)
    b2 = np.asarray(inputs["b2"], dtype=np.float32)
    m2 = np.asarray(inputs["m2"], dtype=np.float32)
    v2 = np.asarray(inputs["v2"], dtype=np.float32)

    def fold(w, g, b, m, v):
        s = np.float32(max(np.median(np.abs(w)), EPS))
        t = np.clip(np.round(w / s), -1.0, 1.0).astype(np.float32)
        inv = g / np.sqrt(v + BN_EPS)
        scale = (s * inv).astype(np.float32)
        bias = (b - m * inv).astype(np.float32)
        return t, scale, bias

    t1, sc1, bi1 = fold(w1, g1, b1, m1, v1)
    t2, sc2, bi2 = fold(w2, g2, b2, m2, v2)

    Wa, Wb, Wc, Wd = (t2[:, i * HID:(i + 1) * HID] for i in range(4))

    # Calibration: per-channel means of h, y1, y2, y3 from 2 images,
    # using the kernel's mixed fp8/bf16 x quantization.
    nb = x.shape[0]
    cal_imgs = [0, nb // 2] if nb > 1 else [0]
    cals = []
    for bi_ in cal_imgs:
        xb = x[bi_].reshape(C1, S)
        xq = np.empty_like(xb)
        xq[:NFP8] = xb[:NFP8].astype(NPFP8).astype(np.float32)
        xq[NFP8:] = xb[NFP8:].astype(NPBF16).astype(np.float32)
        ps1 = t1 @ xq
        pre = sc1[:, None] * ps1 + bi1[:, None]
        h = (pre / (1.0 + np.exp(-pre))).astype(NPBF16).astype(np.float32)
        y1 = _maxpool5_np(h.reshape(HID, H, W))
        y2 = _maxpool5_np(y1)
        y3 = _maxpool5_np(y2)
        cals.append([h.mean(axis=1), y1.reshape(HID, S).mean(axis=1),
                     y2.reshape(HID, S).mean(axis=1),
                     y3.reshape(HID, S).mean(axis=1)])
    ch, cy1, cy2_, cy3_ = (
        np.mean([c[i] for c in cals], axis=0).astype(np.float32)
        for i in range(4)
    )

    # bias correction: cv2 sees centered blocks, so add back W@c
    corr = Wa @ ch + Wb @ cy1 + Wc @ cy2_ + Wd @ cy3_
    bi2e = (bi2 + sc2 * corr).astype(np.float32)

    # negated centering constants, packed per V8 k-subtile [16*128]
    cneg = np.concatenate([-ch, -cy1, -cy2_, -cy3_]).astype(np.float32)

    # p-major layouts so each SBUF partition's data is contiguous in DRAM
    # (large DMA descriptors): weights [128, kt, m], x [b, half, 128, kt, 800].
    w1t = t1.T
    w1t8 = np.ascontiguousarray(
        w1t[:NFP8].reshape(KT8, 128, HID).transpose(1, 0, 2)).astype(NPFP8)
    w1t16 = np.ascontiguousarray(
        w1t[NFP8:].reshape(KT16, 128, HID).transpose(1, 0, 2)).astype(NPBF16)
    w2t = np.ascontiguousarray(
        t2.T.reshape(KT2, 128, C2).transpose(1, 0, 2)).astype(NPFP8)

    nb_ = x.shape[0]
    xr = x.reshape(nb_, C1, S)
    xq8 = np.ascontiguousarray(
        xr[:, :NFP8, :].reshape(nb_, KT8, 128, NQ, QW)
        .transpose(0, 3, 2, 1, 4)).astype(NPFP8)
    xq16 = np.ascontiguousarray(
        xr[:, NFP8:, :].reshape(nb_, KT16, 128, NQ, QW)
        .transpose(0, 3, 2, 1, 4)).astype(NPBF16)
    shared = dict(w1t8=w1t8, w1t16=w1t16, w2t=w2t, sc1=sc1, bi1=bi1,
                  sc2=sc2, bi2=bi2e, cng=cneg)
    in_maps = []
    for d in range(N_CORES):
        m = dict(shared)
        m["xq8"] = np.ascontiguousarray(xq8[d * BL:(d + 1) * BL])
        m["xq16"] = np.ascontiguousarray(xq16[d * BL:(d + 1) * BL])
        in_maps.append(m)
    return in_maps


def _install_ntff_hook():
    """The agent image's antenv lacks axon_hooks; synthesize it so
    run_bass_kernel_spmd(trace=True) can capture NTFF profiles via the
    axon .so's C ABI (same mechanism trn_boot would install)."""
    import types

    try:
        import antenv.axon_hooks  # noqa: F401

        return
    except ImportError:
        pass
    try:
        import antenv

        bootdir = "/root/.axon_site/trn_agent_boot"
        if bootdir not in sys.path and os.path.isdir(bootdir):
            sys.path.insert(0, bootdir)
        import trn_boot

        hook = trn_boot._ntff_profile_via_ctypes("/opt/axon/libaxon_pjrt.so")
        mod = types.ModuleType("antenv.axon_hooks")
        state = {"h": hook}
        mod.get_axon_ntff_profile_hook = lambda: state["h"]
        mod.set_axon_ntff_profile_hook = lambda h: state.update(h=h)
        sys.modules["antenv.axon_hooks"] = mod
        antenv.axon_hooks = mod
    except Exception as e:  # profiling is best-effort; execution still works
        print(f"ntff hook install failed: {e}", file=sys.stderr)


def _run(inputs, trace=False):
    from concourse import bass_utils

    if trace:
        _install_ntff_hook()
    nc = _get_nc()
    in_maps = _prep(inputs)
    import time

    res = None
    for attempt, delay in ((0, 5), (1, 20), (2, 0)):
        try:
            res = bass_utils.run_bass_kernel_spmd(
                nc, in_maps, core_ids=list(range(N_CORES)), trace=trace,
            )
            break
        except Exception as e:  # transient device errors happen; back off
            if attempt == 2:
                raise
            print(
                f"run_bass_kernel_spmd failed ({type(e).__name__}); "
                f"retrying in {delay}s",
                file=sys.stderr,
            )
            time.sleep(delay)
    assert res is not None
    outs = [res.results[d]["out"] for d in range(N_CORES)]
    full = np.concatenate(outs, axis=0).reshape(B, C2, H, W).astype(np.float32)
    return full, res


def kernel(**inputs):
    full, _ = _run(inputs, trace=False)
    return full


def run_traced(**inputs):
    full, res = _run(inputs, trace=True)
    return full, res.exec_time_ns
